# revision 8
# baseline (speedup 1.0000x reference)
"""Multi-head attention (B=2, L=2048, D=1024, H=16, DK=64) on 8 TRN2 NeuronCores.

Sharding: core c handles batch b = c//4 and head-group g = c%4 (4 heads,
256 model dims). Per-core compute (no collectives):
  QT/KT  [256, 2048] = wqT_g/wkT_g^T-contracted projections in [dk, seq] layout
  V      [2048, 256] with a fused ones-column per head (gives softmax Z for free)
  S^T    = K_h @ Q_h^T per head in [keys, queries] layout (row-packed head pairs)
  P      = exp(S^T / 8)          (no max subtraction: |scores| <= ~10)
  ctx^T  = V'_h^T @ P            -> rows 0:64 ctx, row 64 = Z
  ctx    /= Z                    (DMA partition-broadcast of 1/Z + DVE mul)
  out_g  = ctx_g @ w_o[:, g]^T   -> per-core PARTIAL output [2048, 1024]
Host sums the 4 head-group partials per batch and stacks the 2 batches.
"""

import numpy as np

D = 1024
L = 2048
DK = 64
GH = 4           # heads per core
GD = GH * DK     # model dims per core (256)
NCORES = 8

MM_DTYPE = "float32r"   # matmul operand dtype: float32r | float32 | bfloat16


def _build(mm_dtype: str = MM_DTYPE):
    import concourse.bacc as bacc
    import concourse.mybir as mybir
    import concourse.tile as tile

    f32 = mybir.dt.float32
    mmdt = getattr(mybir.dt, mm_dtype)
    Exp = mybir.ActivationFunctionType.Exp

    nc = bacc.Bacc("TRN2", target_bir_lowering=False, debug=False,
                   num_devices=NCORES)
    xT = nc.dram_tensor("xT", [D, L], f32, kind="ExternalInput").ap()
    wqT = nc.dram_tensor("wqT", [D, GD], f32, kind="ExternalInput").ap()
    wkT = nc.dram_tensor("wkT", [D, GD], f32, kind="ExternalInput").ap()
    wvT = nc.dram_tensor("wvT", [D, GD], f32, kind="ExternalInput").ap()
    woT = nc.dram_tensor("woT", [GD, D], f32, kind="ExternalInput").ap()
    out = nc.dram_tensor("out", [L, D], f32, kind="ExternalOutput").ap()

    mdt = mmdt           # dtype for matmul-feeding SBUF tiles
    def dr(t):           # reinterpret a DRAM f32 AP as the matmul dtype for DMA
        return t.bitcast(mmdt) if mm_dtype == "float32r" else t

    ND = D // 128    # 8 d-tiles
    NL = L // 128    # 16 key tiles
    NQ = L // 512    # 4 query chunks

    with tile.TileContext(nc) as tc:
        with (
            tc.tile_pool(name="xp", bufs=1) as xp,
            tc.tile_pool(name="wp", bufs=1) as wp,
            tc.tile_pool(name="qk", bufs=1) as qk,
            tc.tile_pool(name="vp", bufs=1) as vp,
            tc.tile_pool(name="cx", bufs=1) as cx,
            tc.tile_pool(name="pp", bufs=6) as pp,
            tc.tile_pool(name="rp", bufs=4) as rp,
            tc.tile_pool(name="op", bufs=3) as op_,
            tc.tile_pool(name="ps", bufs=4, space="PSUM") as ps,
            tc.tile_pool(name="pc", bufs=2, space="PSUM") as pc,
        ):
            # ---- DMA inputs -------------------------------------------------
            wqts = wp.tile([128, ND, GD], mdt, tag="wq", name="wq")
            wkts = wp.tile([128, ND, GD], mdt, tag="wk", name="wk")
            wvts = wp.tile([128, ND, GD], mdt, tag="wv", name="wv")
            wots = [wp.tile([128, D], mdt, tag=f"wo{i}", name=f"wo{i}") for i in range(2)]
            for w_dram, w_sb in ((wqT, wqts), (wkT, wkts)):
                r = w_dram.rearrange("(d p) c -> p d c", p=128)
                for i in range(4):
                    nc.sync.dma_start(w_sb[:, 2 * i:2 * i + 2, :],
                                      dr(r[:, 2 * i:2 * i + 2, :]))
            xts = [xp.tile([128, L], mdt, tag=f"x{d}", name=f"x{d}") for d in range(ND)]
            for d in range(ND):
                for hhalf in range(2):
                    sl = slice(hhalf * 1024, (hhalf + 1) * 1024)
                    nc.sync.dma_start(xts[d][:, sl],
                                      dr(xT[d * 128:(d + 1) * 128, sl]))
            r = wvT.rearrange("(d p) c -> p d c", p=128)
            for i in range(4):
                nc.sync.dma_start(wvts[:, 2 * i:2 * i + 2, :],
                                  dr(r[:, 2 * i:2 * i + 2, :]))
            for i in range(2):
                nc.sync.dma_start(wots[i][:], dr(woT[i * 128:(i + 1) * 128, :]))

            # ---- projections ------------------------------------------------
            qth = [qk.tile([128, L], mdt, tag=f"q{hp}", name=f"q{hp}") for hp in range(2)]
            kth = [qk.tile([128, L], mdt, tag=f"k{hp}", name=f"k{hp}") for hp in range(2)]
            for hp in range(2):
                for w_sb, dstl in ((wqts, qth), (wkts, kth)):
                    for qc in range(NQ):
                        acc = ps.tile([128, 512], f32, tag="s", name="s")
                        for d in range(ND):
                            nc.tensor.matmul(
                                acc[:],
                                w_sb[:, d, hp * 128:(hp + 1) * 128],
                                xts[d][:, qc * 512:(qc + 1) * 512],
                                start=(d == 0), stop=(d == ND - 1))
                        nc.vector.tensor_copy(
                            dstl[hp][:, qc * 512:(qc + 1) * 512], acc[:])

            # V with ones column per head: vph[h] is [keys 128, lt, 65]
            vph = [vp.tile([128, NL, DK + 1], mdt, tag=f"v{h}", name=f"v{h}")
                   for h in range(GH)]
            onesc = wp.tile([128, NL, 1], f32, tag="ones", name="ones")
            nc.vector.memset(onesc[:], 1.0)
            for h in range(GH):
                nc.vector.tensor_copy(vph[h][:, :, DK:DK + 1], onesc[:])
            for lt in range(NL):
                acc = ps.tile([128, GD], f32, tag="s", name="s")
                for d in range(ND):
                    nc.tensor.matmul(
                        acc[:],
                        xts[d][:, lt * 128:(lt + 1) * 128],
                        wvts[:, d, :],
                        start=(d == 0), stop=(d == ND - 1))
                for h in range(GH):
                    nc.vector.tensor_copy(
                        vph[h][:, lt, 0:DK], acc[:, h * DK:(h + 1) * DK])

            # ---- attention + output projection ------------------------------
            ctxt = [cx.tile([128, L], mdt, tag=f"c{hp}", name=f"c{hp}") for hp in range(2)]
            for qc in range(NQ):
                qsl = slice(qc * 512, (qc + 1) * 512)
                for hp in range(2):
                    cps = [pc.tile([DK + 1, 512], f32, tag=f"c{i}", name=f"c{i}")
                           for i in range(2)]
                    for lt in range(NL):
                        lsl = slice(lt * 128, (lt + 1) * 128)
                        ptiles = []
                        for i in range(2):
                            row = slice(i * 64, (i + 1) * 64)
                            sp = ps.tile([128, 512], f32, tag="s", name="s")
                            nc.tensor.matmul(
                                sp[:], kth[hp][row, lsl],
                                qth[hp][row, qsl],
                                start=True, stop=True,
                                tile_position=(i * 64, 0))
                            p = pp.tile([128, 512], mdt, tag=f"p{i}", name=f"p{i}")
                            nc.scalar.activation(p[:], sp[:], Exp, scale=0.125)
                            ptiles.append(p)
                        for i in range(2):
                            nc.tensor.matmul(
                                cps[i][:], vph[2 * hp + i][:, lt, :],
                                ptiles[i][:],
                                start=(lt == 0), stop=(lt == NL - 1))
                    # normalize: ctx /= Z  (row DK of cps is Z)
                    for i in range(2):
                        rz = rp.tile([1, 512], f32, tag="rz", name="rz")
                        nc.vector.reciprocal(rz[:], cps[i][DK:DK + 1, :])
                        rzb = rp.tile([64, 512], f32, tag="rzb", name="rzb")
                        nc.gpsimd.partition_broadcast(rzb[:], rz[0:1, :])
                        nc.vector.tensor_mul(
                            ctxt[hp][i * 64:(i + 1) * 64, qsl],
                            cps[i][0:DK, :], rzb[:])
                # out projection for this query chunk
                for qt in range(4):
                    rows = slice(qc * 512 + qt * 128, qc * 512 + (qt + 1) * 128)
                    for ec in range(2):
                        esl = slice(ec * 512, (ec + 1) * 512)
                        po = ps.tile([128, 512], f32, tag="s", name="s")
                        for hp in range(2):
                            nc.tensor.matmul(
                                po[:], ctxt[hp][:, rows],
                                wots[hp][:, esl],
                                start=(hp == 0), stop=(hp == 1))
                        ot = op_.tile([128, 512], f32, tag="ot", name="ot")
                        nc.vector.tensor_copy(ot[:], po[:])
                        nc.sync.dma_start(out[rows, esl], ot[:])
    nc.compile()
    return nc


_CACHED = {}


def _get_nc(mm_dtype: str = MM_DTYPE):
    if mm_dtype not in _CACHED:
        _CACHED[mm_dtype] = _build(mm_dtype)
    return _CACHED[mm_dtype]


def _round_fp32r(a):
    """Round-to-nearest-even fp32 -> fp32r (11 explicit mantissa bits)."""
    u = np.ascontiguousarray(a, np.float32).view(np.uint32).copy()
    u += 0x7FF + ((u >> 12) & 1)
    u &= 0xFFFFF000
    return u.view(np.float32)


def make_in_maps(x, w_qkv, w_o):
    rnd = _round_fp32r if MM_DTYPE == "float32r" else (lambda a: a)
    x, w_qkv, w_o = rnd(x.reshape(-1)).reshape(x.shape), rnd(w_qkv), rnd(w_o)
    wq, wk, wv = (w_qkv[i * D:(i + 1) * D] for i in range(3))
    in_maps = []
    for c in range(NCORES):
        b, g = divmod(c, 4)
        gs = slice(g * GD, (g + 1) * GD)
        in_maps.append({
            "xT": np.ascontiguousarray(x[b].T),
            "wqT": np.ascontiguousarray(wq[gs].T),
            "wkT": np.ascontiguousarray(wk[gs].T),
            "wvT": np.ascontiguousarray(wv[gs].T),
            "woT": np.ascontiguousarray(w_o[:, gs].T),
        })
    return in_maps


def assemble(results):
    out = np.empty((2, L, D), np.float32)
    for b in range(2):
        out[b] = sum(results[4 * b + g]["out"] for g in range(4))
    return out


def kernel(x, w_qkv, w_o):
    from concourse import bass_utils
    nc = _get_nc()
    in_maps = make_in_maps(np.asarray(x, np.float32),
                           np.asarray(w_qkv, np.float32),
                           np.asarray(w_o, np.float32))
    res = bass_utils.run_bass_kernel_spmd(
        nc, in_maps, core_ids=list(range(NCORES)))
    return assemble(res.results)


# revision 11
# speedup vs baseline: 1.5200x; 1.5200x over previous
"""Multi-head attention (B=2, L=2048, D=1024, H=16, DK=64) on 8 TRN2 NeuronCores.

Sharding: core c handles batch b = c//4 and head-group g = c%4 (4 heads,
256 model dims). Per-core compute (no collectives):
  QT/KT  [256, 2048] projections in [dk, seq] layout (rhs = x^T, lhsT = w^T)
  V      [2048, 256] with a fused ones-column per head (gives softmax Z free)
  S^T    = K_h @ Q_h^T per head in [keys, queries] layout (row-packed head
           pairs on the PE array, K=64 each)
  P      = exp(S^T / 8)     one wide ACT op per (key-tile, head-pair)
  ctx^T  = V'_h^T @ P       -> rows 0:64 ctx, row 64 = Z
  ctx   /= Z                (batched reciprocal + gpsimd partition broadcast)
  out_g  = ctx_g @ w_o[:, g]^T   -> per-core PARTIAL output [2048, 1024]
Host sums the 4 head-group partials per batch and stacks the 2 batches.
"""

import numpy as np

D = 1024
L = 2048
DK = 64
GH = 4           # heads per core
GD = GH * DK     # model dims per core (256)
NCORES = 8

MM_DTYPE = "bfloat16"   # matmul operand dtype: bfloat16 | float32r | float32


def _build(mm_dtype: str = MM_DTYPE):
    import concourse.bacc as bacc
    import concourse.mybir as mybir
    import concourse.tile as tile

    f32 = mybir.dt.float32
    mmdt = getattr(mybir.dt, mm_dtype)
    Exp = mybir.ActivationFunctionType.Exp

    # DRAM input dtype: bf16 ships converted data; f32r ships f32 bits
    # (pre-rounded on host) and bitcasts the DMA source AP.
    ddt = mmdt if mm_dtype == "bfloat16" else f32

    nc = bacc.Bacc("TRN2", target_bir_lowering=False, debug=False,
                   num_devices=NCORES)
    xT = nc.dram_tensor("xT", [D, L], ddt, kind="ExternalInput").ap()
    wqT = nc.dram_tensor("wqT", [D, GD], ddt, kind="ExternalInput").ap()
    wkT = nc.dram_tensor("wkT", [D, GD], ddt, kind="ExternalInput").ap()
    wvT = nc.dram_tensor("wvT", [D, GD], ddt, kind="ExternalInput").ap()
    woT = nc.dram_tensor("woT", [GD, D], ddt, kind="ExternalInput").ap()
    out = nc.dram_tensor("out", [L, D], f32, kind="ExternalOutput").ap()

    mdt = mmdt           # dtype for matmul-feeding SBUF tiles

    def dr(t):           # reinterpret a DRAM f32 AP as the matmul dtype
        return t.bitcast(mmdt) if mm_dtype == "float32r" else t

    ND = D // 128    # 8 d-tiles
    NL = L // 128    # 16 key tiles
    NQ = L // 512    # 4 query chunks

    with tile.TileContext(nc) as tc:
        with (
            tc.tile_pool(name="xp", bufs=1) as xp,
            tc.tile_pool(name="wp", bufs=1) as wp,
            tc.tile_pool(name="qk", bufs=1) as qk,
            tc.tile_pool(name="vp", bufs=1) as vp,
            tc.tile_pool(name="cx", bufs=1) as cx,
            tc.tile_pool(name="pp", bufs=6) as pp,
            tc.tile_pool(name="rp", bufs=4) as rp,
            tc.tile_pool(name="op", bufs=4) as op_,
            tc.tile_pool(name="ps", bufs=2, space="PSUM") as ps,
            tc.tile_pool(name="pc", bufs=2, space="PSUM") as pc,
        ):
            # ---- DMA inputs -------------------------------------------------
            wqts = wp.tile([128, ND, GD], mdt, tag="wq", name="wq")
            wkts = wp.tile([128, ND, GD], mdt, tag="wk", name="wk")
            wvts = wp.tile([128, ND, GD], mdt, tag="wv", name="wv")
            wots = [wp.tile([128, D], mdt, tag=f"wo{i}", name=f"wo{i}")
                    for i in range(2)]
            for w_dram, w_sb in ((wqT, wqts), (wkT, wkts)):
                r = w_dram.rearrange("(d p) c -> p d c", p=128)
                for i in range(4):
                    nc.sync.dma_start(w_sb[:, 2 * i:2 * i + 2, :],
                                      dr(r[:, 2 * i:2 * i + 2, :]))
            xts = [xp.tile([128, L], mdt, tag=f"x{d}", name=f"x{d}")
                   for d in range(ND)]
            for d in range(ND):
                for hhalf in range(2):
                    sl = slice(hhalf * 1024, (hhalf + 1) * 1024)
                    nc.sync.dma_start(xts[d][:, sl],
                                      dr(xT[d * 128:(d + 1) * 128, sl]))
            r = wvT.rearrange("(d p) c -> p d c", p=128)
            for i in range(4):
                nc.sync.dma_start(wvts[:, 2 * i:2 * i + 2, :],
                                  dr(r[:, 2 * i:2 * i + 2, :]))
            for i in range(2):
                nc.sync.dma_start(wots[i][:], dr(woT[i * 128:(i + 1) * 128, :]))

            # ---- projections ------------------------------------------------
            qth = [qk.tile([128, L], mdt, tag=f"q{hp}", name=f"q{hp}")
                   for hp in range(2)]
            kth = [qk.tile([128, L], mdt, tag=f"k{hp}", name=f"k{hp}")
                   for hp in range(2)]
            for hp in range(2):
                for w_sb, dstl in ((wqts, qth), (wkts, kth)):
                    for qc in range(NQ):
                        acc = ps.tile([128, 1024], f32, tag="s", name="s")
                        for d in range(ND):
                            nc.tensor.matmul(
                                acc[:, 0:512],
                                w_sb[:, d, hp * 128:(hp + 1) * 128],
                                xts[d][:, qc * 512:(qc + 1) * 512],
                                start=(d == 0), stop=(d == ND - 1))
                        nc.vector.tensor_copy(
                            dstl[hp][:, qc * 512:(qc + 1) * 512], acc[:, 0:512])

            # V with ones column per head: vph[h] is [keys 128, lt, 65]
            vph = [vp.tile([128, NL, DK + 1], mdt, tag=f"v{h}", name=f"v{h}")
                   for h in range(GH)]
            onesc = wp.tile([128, NL, 1], f32, tag="ones", name="ones")
            nc.vector.memset(onesc[:], 1.0)
            for h in range(GH):
                nc.vector.tensor_copy(vph[h][:, :, DK:DK + 1], onesc[:])
            for lt in range(NL):
                acc = ps.tile([128, 1024], f32, tag="s", name="s")
                for d in range(ND):
                    nc.tensor.matmul(
                        acc[:, 0:GD],
                        xts[d][:, lt * 128:(lt + 1) * 128],
                        wvts[:, d, :],
                        start=(d == 0), stop=(d == ND - 1))
                for h in range(GH):
                    nc.vector.tensor_copy(
                        vph[h][:, lt, 0:DK], acc[:, h * DK:(h + 1) * DK])

            # ---- attention + output projection ------------------------------
            ctxt = [cx.tile([128, L], mdt, tag=f"c{hp}", name=f"c{hp}")
                    for hp in range(2)]
            for qc in range(NQ):
                qsl = slice(qc * 512, (qc + 1) * 512)
                zq = rp.tile([128, 512], f32, tag="zq", name="zq", bufs=2)
                nc.vector.memset(zq[:], 1.0)
                cps_all = []
                for hp in range(2):
                    cps = [pc.tile([DK + 1, 512], f32, tag=f"c{i}",
                                   name=f"c{i}") for i in range(2)]
                    cps_all.append(cps)
                    for lt in range(NL):
                        lsl = slice(lt * 128, (lt + 1) * 128)
                        sp = ps.tile([128, 1024], f32, tag="s", name="s")
                        for i in range(2):
                            row = slice(i * 64, (i + 1) * 64)
                            nc.tensor.matmul(
                                sp[:, i * 512:(i + 1) * 512],
                                kth[hp][row, lsl], qth[hp][row, qsl],
                                start=True, stop=True,
                                tile_position=(i * 64, 0))
                        p = pp.tile([128, 1024], mdt, tag="p", name="p")
                        nc.scalar.activation(p[:], sp[:], Exp, scale=0.125)
                        for i in range(2):
                            nc.tensor.matmul(
                                cps[i][:], vph[2 * hp + i][:, lt, :],
                                p[:, i * 512:(i + 1) * 512],
                                start=(lt == 0), stop=(lt == NL - 1))
                    for i in range(2):
                        zrow = 32 * (2 * hp + i)
                        nc.vector.tensor_copy(
                            zq[zrow:zrow + 1, :], cps[i][DK:DK + 1, :])
                rzq = rp.tile([128, 512], f32, tag="rzq", name="rzq", bufs=2)
                nc.vector.reciprocal(rzq[0:128, :], zq[0:128, :])
                for hp in range(2):
                    for i in range(2):
                        zrow = 32 * (2 * hp + i)
                        rzi = rp.tile([1, 512], f32, tag="rzi", name="rzi")
                        nc.vector.tensor_copy(rzi[:], rzq[zrow:zrow + 1, :])
                        rzb = rp.tile([64, 512], f32, tag="rzb", name="rzb")
                        nc.gpsimd.partition_broadcast(rzb[:], rzi[:])
                        nc.vector.tensor_mul(
                            ctxt[hp][i * 64:(i + 1) * 64, qsl],
                            cps_all[hp][i][0:DK, :], rzb[:])
                # out projection for this query chunk
                for qt in range(4):
                    rows = slice(qc * 512 + qt * 128,
                                 qc * 512 + (qt + 1) * 128)
                    for ec in range(2):
                        esl = slice(ec * 512, (ec + 1) * 512)
                        po = ps.tile([128, 1024], f32, tag="s", name="s")
                        for hp in range(2):
                            nc.tensor.matmul(
                                po[:, 0:512], ctxt[hp][:, rows],
                                wots[hp][:, esl],
                                start=(hp == 0), stop=(hp == 1))
                        ot = op_.tile([128, 512], f32, tag="ot", name="ot")
                        nc.vector.tensor_copy(ot[:], po[:, 0:512])
                        nc.sync.dma_start(out[rows, esl], ot[:])
    nc.compile()
    return nc


_CACHED = {}


def _get_nc(mm_dtype: str = MM_DTYPE):
    if mm_dtype not in _CACHED:
        _CACHED[mm_dtype] = _build(mm_dtype)
    return _CACHED[mm_dtype]


def _round_fp32r(a):
    """Round-to-nearest-even fp32 -> fp32r (11 explicit mantissa bits)."""
    u = np.ascontiguousarray(a, np.float32).view(np.uint32).copy()
    u += 0x7FF + ((u >> 12) & 1)
    u &= 0xFFFFF000
    return u.view(np.float32)


def make_in_maps(x, w_qkv, w_o):
    if MM_DTYPE == "float32r":
        cvt = _round_fp32r
    elif MM_DTYPE == "bfloat16":
        import ml_dtypes
        cvt = lambda a: np.asarray(a, dtype=ml_dtypes.bfloat16)  # noqa: E731
    else:
        cvt = lambda a: a  # noqa: E731
    wq, wk, wv = (w_qkv[i * D:(i + 1) * D] for i in range(3))
    in_maps = []
    for c in range(NCORES):
        b, g = divmod(c, 4)
        gs = slice(g * GD, (g + 1) * GD)
        in_maps.append({
            "xT": cvt(np.ascontiguousarray(x[b].T)),
            "wqT": cvt(np.ascontiguousarray(wq[gs].T)),
            "wkT": cvt(np.ascontiguousarray(wk[gs].T)),
            "wvT": cvt(np.ascontiguousarray(wv[gs].T)),
            "woT": cvt(np.ascontiguousarray(w_o[:, gs].T)),
        })
    return in_maps


def assemble(results):
    out = np.empty((2, L, D), np.float32)
    for b in range(2):
        out[b] = sum(results[4 * b + g]["out"] for g in range(4))
    return out


def kernel(x, w_qkv, w_o):
    from concourse import bass_utils
    nc = _get_nc()
    in_maps = make_in_maps(np.asarray(x, np.float32),
                           np.asarray(w_qkv, np.float32),
                           np.asarray(w_o, np.float32))
    res = bass_utils.run_bass_kernel_spmd(
        nc, in_maps, core_ids=list(range(NCORES)))
    return assemble(res.results)
